# revision 68
# baseline (speedup 1.0000x reference)
"""Trainium2 Bass kernel for nn_Attention: fused QKV + RoPE + softmax attention + o_proj.

Sharding (8 cores): core c -> (batch b = c//2, head-half g = c%2).
Each core computes 8 of 16 heads for one batch; host sums the two
head-half partials per batch and transposes.

Design (driven by the TimelineSim cost model, which charges a matmul
output_free_size x pe_cycle and gives full PE clock only after ~3us of
continuous execution):
  - one global stream of 256 blocks = 16 (chunk, pair) units x 16 kpos
    tiles; scores [128 kpos, 512 q] per pair (two tile_position-packed
    64-contraction matmuls) lead the ACT exp stream by 2 blocks through a
    2-tile psum ring; exp is the only ACT work (256 x free-1024 tiles).
  - P@V with E stationary: per (head, q-block) one psum accumulation
    group [128 q, 65] over all 16 kpos tiles (moving = 64 V columns + a
    ones column -> denominator lands in column 64), charging 65 rows
    instead of 512 per matmul.  One group per bank at a time (hardware
    zeroes the full 2KB region on start); groups ping-pong 2 psum banks,
    trailing each unit's last exp via a 22-deep e-tile ring.
  - normalize on DVE (reciprocal + per-partition tensor_scalar mult),
    then SBUF->SBUF DMA transpose (XBAR) assembles attnT [vf, q] with no
    PE or extra psum cost; o_proj consumes attnT per chunk.
  - K/V/Q projections and o_proj are background items placed by a
    latest-safe-fit backward scheduler (deadline = first consuming
    block); projections split into two half-contraction phases so a
    block's PE spike stays under the exp cadence.  Unit order is
    pair-major, spreading each pair's K-projection deadline evenly.
  - PE warmup matmuls bridge the DMA prologue so the p-state ramp is
    complete when real work starts; DMA transfers are few, large, and
    priority-ordered (HWDGE issues one DMA per ~625ns regardless of
    size).
PSUM banks: scores 2x2 + P@V 2 + projection ping-pong 2 = 8.
"""
import os
import sys

sys.path.insert(0, "/opt/trn_rl_repo")

import numpy as np
import ml_dtypes

import concourse.bass as bass
import concourse.mybir as mybir
import concourse.tile as tile
from concourse.bass_utils import run_bass_kernel_spmd
from concourse.vector_clock import ScopedClock, VectorClock

# ---------------------------------------------------------------------------
# Patch TileContext._drain_and_barrier: the walrus build in this container
# allows only ONE sync-wait per instruction; Tile's tail drain carries one
# wait per active proc.  Split them into single-wait NOPs on SP.
N_PROCS = 27


def _patched_drain_and_barrier(self, tick_clock, wait_clock):
    nc = self.nc
    gc = tick_clock.global_clock
    for p in range(N_PROCS):
        t = gc[p]
        if t > 0:
            nop = nc.sync.nop(nofuse=True)
            vc = VectorClock([t if q == p else 0 for q in range(N_PROCS)])
            wait_clock.add_sem_waits(nop.ins, ScopedClock({None: vc}))
    nc.sync.drain()
    nc.all_engine_barrier()
    assert self.sems is not None
    popped = nc._tile_sem_poison_stack.pop()
    assert popped is self._sem_poison
    nc.clear_and_free_semaphores(list(self.sems.allocated().values()))
    nc.all_engine_barrier()


tile.TileContext._drain_and_barrier = _patched_drain_and_barrier


def _split_excess_waits(nc):
    """walrus in this container accepts 1 sync-wait per instruction (2 on
    EventSemaphore).  Move excess waits onto EventSemaphore instructions
    inserted just before, on the same engine."""
    for f in nc.m.functions:
        for bb in f.blocks:
            new_insts = []
            changed = False
            for ins in bb.instructions:
                si = ins.sync_info
                waits = list(si.on_wait) if si is not None else []
                cap = 2 if isinstance(ins, mybir.InstEventSemaphore) else 1
                if len(waits) > cap:
                    changed = True
                    excess = waits[: len(waits) - cap]
                    for i in range(0, len(excess), 2):
                        ev = mybir.InstEventSemaphore(
                            name=f"I-{nc.next_id()}",
                            engine=ins.engine,
                            ins=[],
                            outs=[],
                            sync_info=mybir.SyncInfo(
                                on_wait=excess[i : i + 2], on_update=[]
                            ),
                        )
                        nc.register_instruction(ev)
                        new_insts.append(ev)
                    si.on_wait = waits[len(waits) - cap :]
                new_insts.append(ins)
            if changed:
                bb.instructions[:] = new_insts
# ---------------------------------------------------------------------------

B, S, H, NH, HD = 4, 2048, 1024, 16, 64
HPC = NH // 2          # heads per core
PAIRS = HPC // 2       # head pairs per core
HT = H // 128          # hidden-dim tiles
QKF = 2 * HPC * HD     # q+k features per core (1024)
VF = HPC * HD          # v features per core (512)
SC = 512               # seq chunk (psum bank pair)
NSC = S // SC
KT = S // 128          # kpos tiles
QB = SC // 128         # q blocks per chunk (4)
BF = mybir.dt.float16
F32 = mybir.dt.float32
EXP_SCALE = 1.0 / float(np.sqrt(HD))

# Pipeline knobs
EPS_BUFS = 22          # e-tile ring; P@V groups of unit u read e(u, kt 0..15)
                       # and must finish before exp reuses those bufs
PV_PER_BLOCK = int(__import__("os").environ.get("ATTN_PPB", "2"))       # P@V groups emitted per block after a unit's last exp
PV_OFF = 1             # delay P@V groups one block past the unit boundary

# Unit order: (qc, pair) pair-streams, pair-major: each pair streams all four
# chunks before the next pair starts, so K(p) projections spread evenly at
# blocks 0/64/128/192 instead of piling into the front window.
import os as _os
_ORDER = _os.environ.get("ATTN_ORDER", "pair")
if _ORDER == "pair":
    UNITS = [(qc, p) for p in range(4) for qc in range(4)]
elif _ORDER == "chunk2":
    UNITS = [(qc, p) for grp in (0, 1) for p in range(4)
             for qc in (2 * grp, 2 * grp + 1)]
elif _ORDER == "chunk3":
    UNITS = ([(qc, p) for p in range(4) for qc in (0, 1, 2)]
             + [(3, p) for p in range(4)])
elif _ORDER == "stag":
    UNITS = [
        (0, 0), (1, 0), (2, 0), (0, 1),
        (1, 1), (0, 2), (1, 2), (0, 3),
        (1, 3), (2, 1), (3, 0), (2, 2),
        (3, 1), (2, 3), (3, 2), (3, 3),
    ]
else:  # chunk-major
    UNITS = [(qc, p) for qc in range(4) for p in range(4)]
NBLK = len(UNITS) * KT  # 256

_CACHED_NC = None


def _build_nc():
    nc = bass.Bass()
    hT = nc.declare_dram_parameter("hT", [128, HT, S], BF, isOutput=False)
    wqk = nc.declare_dram_parameter("wqk", [128, HT, QKF], BF, isOutput=False)
    wv = nc.declare_dram_parameter("wv", [128, HT, VF], BF, isOutput=False)
    wo = nc.declare_dram_parameter("wo", [128, VF // 128, H], BF, isOutput=False)
    cossin = nc.declare_dram_parameter("cossin", [128, 2, S], BF, isOutput=False)
    wqk0 = nc.declare_dram_parameter("wqk0", [128, HT, 256], BF, isOutput=False)
    outT = nc.declare_dram_parameter("outT", [H, S], F32, isOutput=True)

    Exp = mybir.ActivationFunctionType.Exp
    MULT = mybir.AluOpType.mult

    with tile.TileContext(nc) as tc:
        with tc.tile_pool(name="singles", bufs=1) as singles:
            hT_sb = singles.tile([128, HT, S], BF)
            wqk_sb = singles.tile([128, HT, QKF], BF)
            wv_sb = singles.tile([128, HT, VF], BF)
            wo_sb = singles.tile([128, VF // 128, H], BF)
            cossin_sb = singles.tile([128, 2, S], BF)
            wqk0_sb = singles.tile([128, HT, 256], BF)
            q_rope = singles.tile([128, PAIRS, S], BF)
            k_rope = singles.tile([128, PAIRS, S], BF)
            vext = singles.tile([128, KT, HPC * 65], BF)
            nc.gpsimd.memset(vext[:], 1.0)  # ones columns for denominators

            # ---- DMA: few large strided transfers, priority-ordered.
            # HWDGE issues one DMA per ~625ns regardless of size, so
            # consolidate; first-needed data first (first unit is (0, p0)).
            nc.sync.dma_start(out=wqk0_sb[:], in_=wqk0[:])
            nc.sync.dma_start(out=hT_sb[:, :, 0:SC], in_=hT[:, :, 0:SC])
            nc.sync.dma_start(out=cossin_sb[:], in_=cossin[:])
            nc.sync.dma_start(out=wv_sb[:], in_=wv[:])
            nc.sync.dma_start(out=hT_sb[:, :, SC : 2 * SC], in_=hT[:, :, SC : 2 * SC])
            nc.sync.dma_start(
                out=hT_sb[:, :, 2 * SC : 3 * SC], in_=hT[:, :, 2 * SC : 3 * SC]
            )
            nc.sync.dma_start(
                out=hT_sb[:, :, 3 * SC : 4 * SC], in_=hT[:, :, 3 * SC : 4 * SC]
            )
            nc.sync.dma_start(
                out=wqk_sb[:, :, 128 : PAIRS * 128],
                in_=wqk[:, :, 128 : PAIRS * 128],
            )
            nc.sync.dma_start(
                out=wqk_sb[:, :, (PAIRS + 1) * 128 : QKF],
                in_=wqk[:, :, (PAIRS + 1) * 128 : QKF],
            )
            nc.sync.dma_start(out=wo_sb[:], in_=wo[:])

            # ---- pools (PSUM: sps 4 + atps 1 + projps 2 + miscps 1 = 8) ----
            with (
                tc.tile_pool(name="sps", bufs=2, space="PSUM") as sps,
                tc.tile_pool(name="atps", bufs=2, space="PSUM") as atps,
                tc.tile_pool(name="projps", bufs=2, space="PSUM") as projps,
                tc.tile_pool(name="raws", bufs=3) as raws,
                tc.tile_pool(name="ropet", bufs=3) as ropet,
                tc.tile_pool(name="eps", bufs=EPS_BUFS) as eps,
                tc.tile_pool(name="anq", bufs=2) as anq,
                tc.tile_pool(name="recs", bufs=4) as recs,
                tc.tile_pool(name="ant", bufs=4) as ant,
                tc.tile_pool(name="obs", bufs=4) as obs,
            ):
                # PE warmup: the p-state model needs ~3us of continuous PE
                # execution to reach full clock.  Run throwaway matmuls while
                # the first DMAs land so the real stream starts warm.
                dummy_sb = singles.tile([128, SC], BF)
                nc.vector.memset(dummy_sb[:], 0.0)
                wps = projps.tile([128, SC], F32, tag="pj", name="wps")
                for _ in range(int(os.environ.get("ATTN_WARM", "14"))):
                    nc.tensor.matmul(
                        wps[0:1, :],
                        dummy_sb[:, 0:1],
                        dummy_sb[:],
                        start=True,
                        stop=True,
                    )

                pending_proj = {}
                HH = HT // 2  # contraction mms per phase

                def proj_qk(m, c, phase, rope_split=1):
                    """Project q/k feature tile m for seq chunk c, RoPE on DVE.
                    Split into two half-contraction phases to spread PE load."""
                    key = ("qk", m, c)
                    if phase == 0:
                        ps = projps.tile([128, SC], F32, tag="pj", name="pj")
                        pending_proj[key] = ps
                    else:
                        ps = pending_proj.pop(key)
                    if m == 0:
                        wsrc = lambda k: wqk0_sb[:, k, 0:128]
                    elif m == PAIRS:
                        wsrc = lambda k: wqk0_sb[:, k, 128:256]
                    else:
                        wsrc = lambda k: wqk_sb[:, k, m * 128 : (m + 1) * 128]
                    for k in range(phase * HH, (phase + 1) * HH):
                        nc.tensor.matmul(
                            ps[:],
                            wsrc(k),
                            hT_sb[:, k, c * SC : (c + 1) * SC],
                            start=(k == 0),
                            stop=(k == HT - 1),
                        )
                    if phase == 0:
                        return
                    pair = m % PAIRS
                    dst_t = q_rope if m < PAIRS else k_rope
                    raw = raws.tile([128, SC], BF)
                    t1 = ropet.tile([128, SC], BF, tag="t1")
                    t2 = ropet.tile([128, SC], BF, tag="t2")
                    w = SC // rope_split
                    for s in range(rope_split):
                        lo, hi = s * w, (s + 1) * w
                        nc.vector.tensor_copy(raw[:, lo:hi], ps[:, lo:hi])
                        cs = cossin_sb[:, 0, c * SC + lo : c * SC + hi]
                        sn = cossin_sb[:, 1, c * SC + lo : c * SC + hi]
                        dst = dst_t[:, pair, c * SC + lo : c * SC + hi]
                        nc.vector.tensor_mul(t1[:, lo:hi], raw[:, lo:hi], cs)
                        nc.vector.tensor_mul(t2[0:32, lo:hi], raw[32:64, lo:hi], sn[32:64])
                        nc.vector.tensor_mul(t2[32:64, lo:hi], raw[0:32, lo:hi], sn[0:32])
                        nc.vector.tensor_mul(t2[64:96, lo:hi], raw[96:128, lo:hi], sn[96:128])
                        nc.vector.tensor_mul(t2[96:128, lo:hi], raw[64:96, lo:hi], sn[64:96])
                        nc.vector.tensor_add(dst, t1[:, lo:hi], t2[:, lo:hi])

                def proj_v(st, p):
                    """Project V features of pair p for kpos block st."""
                    ps = projps.tile([128, SC], F32, tag="pj", name="pj")
                    for k in range(HT):
                        nc.tensor.matmul(
                            ps[:, 0:128],
                            hT_sb[:, k, st * 128 : (st + 1) * 128],
                            wv_sb[:, k, p * 128 : (p + 1) * 128],
                            start=(k == 0),
                            stop=(k == HT - 1),
                        )
                    vdst = vext[:, st, :].rearrange(
                        "q (h x) -> q h x", x=65
                    )[:, 2 * p : 2 * p + 2, 0:64]
                    vsrc = ps[:, 0:128].rearrange("q (h x) -> q h x", x=64)
                    nc.vector.tensor_copy(vdst, vsrc)

                def emit_scores(g):
                    qc, pair = UNITS[g // KT]
                    kt = g % KT
                    ksl = slice(kt * 128, (kt + 1) * 128)
                    qsl = slice(qc * SC, (qc + 1) * SC)
                    s2 = sps.tile([128, 2, SC], F32, tag="s2", name="s2")
                    nc.tensor.matmul(
                        s2[:, 0, :],
                        k_rope[0:64, pair, ksl],
                        q_rope[0:64, pair, qsl],
                        start=True,
                        stop=True,
                        tile_position=(0, 0),
                    )
                    nc.tensor.matmul(
                        s2[:, 1, :],
                        k_rope[64:128, pair, ksl],
                        q_rope[64:128, pair, qsl],
                        start=True,
                        stop=True,
                        tile_position=(64, 0),
                    )
                    return s2

                def o_proj_m(qc, m):
                    qsl = slice(qc * SC, (qc + 1) * SC)
                    op = projps.tile([128, SC], F32, tag="pj", name="pj")
                    at = attnT_of[qc]
                    for ot in range(VF // 128):
                        nc.tensor.matmul(
                            op[:],
                            wo_sb[:, ot, m * 128 : (m + 1) * 128],
                            at[:, ot, :],
                            start=(ot == 0),
                            stop=(ot == VF // 128 - 1),
                        )
                    ob = obs.tile([128, SC], F32, tag="ob", name="ob")
                    nc.vector.tensor_copy(ob[:], op[:])
                    nc.sync.dma_start(out=outT[m * 128 : (m + 1) * 128, qsl], in_=ob[:])

                # ---------- background work scheduler ----------
                first_step_of_pair = {}
                for i, (qc, p) in enumerate(UNITS):
                    first_step_of_pair.setdefault(p, i * KT)
                unit_start = {u: i * KT for i, u in enumerate(UNITS)}

                bg = []  # (deadline, seq, nphases, kind, args)
                seq = 0
                for p in range(PAIRS):
                    for st in range(KT):
                        # all of pair p's V needed when its first unit's P@V
                        # groups run (block first_step + KT)
                        dl = first_step_of_pair[p] + KT - 4
                        if p == 0:
                            dl = st  # spread the unavoidable front V lump
                        bg.append((max(0, dl), seq, 1, "V", (st, p)))
                        seq += 1
                for p in range(PAIRS):
                    for c in range(NSC):
                        # extra margin on the first chunk: the RoPE chain
                        # must clear the unit-boundary DVE congestion
                        dl = first_step_of_pair[p] + 4 * c - (9 if c == 0 else 5)
                        if p == 0 and c == 0:
                            dl = -1  # prologue: first scores need it
                        else:
                            dl = max(0, dl)
                        bg.append((dl, seq, 2, "K", (p, c))); seq += 1
                for i, (qc, p) in enumerate(UNITS):
                    dl = i * KT - 8
                    if i == 0:
                        dl = -1  # prologue
                    else:
                        dl = max(0, dl)
                    bg.append((dl, seq, 2, "Q", (qc, p))); seq += 1
                bg.sort()
                # Split two-phase projections into half-contraction phases and
                # level backward (latest-safe-fit) so late-deadline work stays
                # in the back half where PE has slack.
                sub = []
                for dl, sq, nph, kind, args in bg:
                    for ph in range(nph):
                        sub.append((dl, sq, ph, kind, args))
                sub.sort(key=lambda t: (t[0], t[1], t[2]), reverse=True)
                bgq = []  # (block, seq, phase, kind, args)
                next_free_back = NBLK - 1
                for dl, sq, phase, kind, args in sub:
                    if dl < 0:
                        blk = -1  # prologue
                    else:
                        blk = max(0, min(dl, next_free_back))
                        next_free_back = blk - 1
                    bgq.append((blk, sq, phase, kind, args))
                bgq.sort(key=lambda t: (t[0], t[1], t[2]))  # phase order safe

                def emit_bg_item(kind, args, phase):
                    if kind == "V":
                        proj_v(*args)
                    elif kind == "K":
                        p, c = args
                        rs = 4 if (p, c) == (0, 0) else 1
                        proj_qk(PAIRS + p, c, phase, rope_split=rs)
                    elif kind == "Q":
                        qc, p = args
                        proj_qk(p, qc, phase)

                # ---------- P@V groups, normalize, transpose ----------
                e_of = {}       # g -> e tile AP
                s2_of = {}      # g -> scores psum AP
                aq_of = {}      # unit idx -> attn_q sbuf AP
                attnT_of = {}   # qc -> attnT sbuf AP

                # group order: both halves of a q-block back-to-back, so
                # its transpose DMA can fire while later groups still run
                GRP_ORDER = [0, 4, 1, 5, 2, 6, 3, 7]

                def emit_pv_group(ui, gi):
                    """One P@V output group: accumulate [128 q, 65] over all
                    kt for (half, qb) = divmod(grp, QB), then normalize."""
                    grp = GRP_ORDER[gi]
                    qc, pair = UNITS[ui]
                    half, qb = divmod(grp, QB)
                    hloc = 2 * pair + half
                    att = atps.tile([128, 65], F32, tag="att", name="att")
                    for kt in range(KT):
                        est = e_of[ui * KT + kt][:, half, qb * 128 : (qb + 1) * 128]
                        vsl = vext[:, kt, hloc * 65 : (hloc + 1) * 65]
                        nc.tensor.matmul(
                            att[:],
                            est,
                            vsl,
                            start=(kt == 0),
                            stop=(kt == KT - 1),
                        )
                    rec = recs.tile([128, 1], F32, tag="rec", name="rec")
                    nc.vector.reciprocal(rec[:], att[:, 64:65])
                    if gi == 0:
                        aq_of[ui] = anq.tile(
                            [128, QB, 128], BF, tag="aq", name="aq"
                        )
                    aq = aq_of[ui]
                    nc.vector.tensor_scalar(
                        aq[:, qb, half * 64 : (half + 1) * 64],
                        att[:, 0:64],
                        rec[:, 0:1],
                        None,
                        MULT,
                    )
                    if half == 1:
                        # both halves of this q-block done: transpose it now
                        if qc not in attnT_of:
                            attnT_of[qc] = ant.tile(
                                [128, PAIRS, SC], BF, tag="at", name="at"
                            )
                        nc.sync.dma_start_transpose(
                            attnT_of[qc][:, pair, qb * 128 : (qb + 1) * 128],
                            aq[:, qb, :],
                        )
                    if gi == 7:
                        for g in range(ui * KT, (ui + 1) * KT):
                            del e_of[g]

                # P@V group schedule: unit ui's groups at blocks
                # ui*KT + KT + i//PV_PER_BLOCK (after its last exp)
                pv_by_block = {}
                for ui in range(len(UNITS)):
                    for i in range(8):
                        blk = ui * KT + KT + PV_OFF + i // PV_PER_BLOCK
                        pv_by_block.setdefault(blk, []).append((ui, i))

                # o_proj items, released once a chunk's attnT is complete
                qc_last_unit = {}
                for i, (qc, p) in enumerate(UNITS):
                    qc_last_unit[qc] = i
                oproj_release = {
                    qc: qc_last_unit[qc] * KT + KT + 8 // PV_PER_BLOCK + 1
                    for qc in range(NSC)
                }
                oproj_q = [(oproj_release[qc], qc, m) for qc in range(NSC) for m in range(HT)]
                oproj_q.sort()

                # ---------- prologue ----------
                bgptr = 0
                prolog = []
                while bgptr < len(bgq) and bgq[bgptr][0] < 0:
                    prolog.append(bgq[bgptr])
                    bgptr += 1
                # Q first: scores(0) needs the full q_rope chunk, while only
                # the first quarter of the K chunk gates it
                prolog.sort(key=lambda t: (t[3] != "Q", t[3] != "K", t[1], t[2]))
                for _, _, phase, kind, args in prolog:
                    emit_bg_item(kind, args, phase)
                s2_of[0] = emit_scores(0)
                s2_of[1] = emit_scores(1)

                # ---------- main loop ----------
                opptr = 0
                last_op = -10
                NTOT = NBLK + KT
                for g in range(NTOT):
                    if g + 2 < NBLK:
                        s2_of[g + 2] = emit_scores(g + 2)
                    if g < NBLK:
                        s2 = s2_of.pop(g)
                        e = eps.tile([128, 2, SC], BF)
                        nc.scalar.activation(
                            out=e[:], in_=s2[:], func=Exp, scale=EXP_SCALE
                        )
                        e_of[g] = e
                    emitted_bg = False
                    while bgptr < len(bgq) and bgq[bgptr][0] <= g:
                        _, _, phase, kind, args = bgq[bgptr]
                        emit_bg_item(kind, args, phase)
                        bgptr += 1
                        emitted_bg = True
                    if (
                        opptr < len(oproj_q)
                        and oproj_q[opptr][0] <= g
                        and (not emitted_bg or g - last_op >= 3)
                    ):
                        _, qc, m = oproj_q[opptr]
                        o_proj_m(qc, m)
                        opptr += 1
                        last_op = g
                    for ui, grp in pv_by_block.get(g, ()):
                        emit_pv_group(ui, grp)

                # ---------- tail ----------
                while opptr < len(oproj_q):
                    _, qc, m = oproj_q[opptr]
                    o_proj_m(qc, m)
                    opptr += 1
    _split_excess_waits(nc)
    return nc


def _prep_inputs(cos, sin, hidden_states, w_qkv, w_o):
    """Per-core host-side sharding/transpose/cast. Returns list of in_maps."""
    bf = np.float16
    cos = np.asarray(cos, np.float32)
    sin = np.asarray(sin, np.float32)
    hidden_states = np.asarray(hidden_states, np.float32)
    w_qkv = np.asarray(w_qkv, np.float32)
    w_o = np.asarray(w_o, np.float32)

    cosT = cos.T  # [64, S]
    cos_t = np.ascontiguousarray(np.tile(cosT, (2, 1))).astype(bf)
    # sin multiplier aligned to the *source* partitions of the rot ops:
    # rows [0:32] = +sin[32:64] (multiplies src q[0:32] -> dest [32:64]),
    # rows [32:64] = -sin[0:32] (multiplies src q[32:64] -> dest [0:32]).
    sinT = sin.T
    sin_t = np.ascontiguousarray(
        np.tile(np.concatenate([sinT[32:], -sinT[:32]], 0), (2, 1))
    ).astype(bf)
    cossin = np.ascontiguousarray(np.stack([cos_t, sin_t], axis=1))

    in_maps = []
    for core in range(8):
        b, g = core // 2, core % 2
        hT = hidden_states[b].T  # [H, S]
        hT_t = np.ascontiguousarray(
            hT.reshape(HT, 128, S).transpose(1, 0, 2)
        ).astype(bf)
        qs, ks, vs = g * VF, NH * HD + g * VF, 2 * NH * HD + g * VF
        wqk_rows = np.concatenate(
            [w_qkv[qs : qs + VF], w_qkv[ks : ks + VF]], 0
        )  # [QKF, H]
        wqk_t = np.ascontiguousarray(
            wqk_rows.T.reshape(HT, 128, QKF).transpose(1, 0, 2)
        ).astype(bf)
        wv_t = np.ascontiguousarray(
            w_qkv[vs : vs + VF].T.reshape(HT, 128, VF).transpose(1, 0, 2)
        ).astype(bf)
        woT = w_o[:, g * VF : (g + 1) * VF].T  # [VF, H]
        wo_t = np.ascontiguousarray(
            woT.reshape(VF // 128, 128, H).transpose(1, 0, 2)
        ).astype(bf)
        wqk0 = np.ascontiguousarray(
            np.concatenate(
                [wqk_t[:, :, 0:128], wqk_t[:, :, PAIRS * 128 : (PAIRS + 1) * 128]],
                axis=2,
            )
        )
        in_maps.append(
            {
                "hT": hT_t,
                "wqk": wqk_t,
                "wqk0": wqk0,
                "wv": wv_t,
                "wo": wo_t,
                "cossin": cossin,
            }
        )
    return in_maps


def kernel(cos, sin, hidden_states, w_qkv, w_o, _trace=False):
    global _CACHED_NC
    if _CACHED_NC is None:
        _CACHED_NC = _build_nc()
    nc = _CACHED_NC
    in_maps = _prep_inputs(cos, sin, hidden_states, w_qkv, w_o)
    res = run_bass_kernel_spmd(nc, in_maps, core_ids=list(range(8)), trace=_trace)
    outs = [r["outT"] for r in res.results]
    out = np.empty((B, S, H), np.float32)
    for b in range(B):
        out[b] = (outs[2 * b] + outs[2 * b + 1]).T
    if _trace:
        return out, res
    return out
